# revision 22
# baseline (speedup 1.0000x reference)
"""CP tensor-regression-layer kernel for Trainium2 (8 NeuronCores).

Computation (matches the reference einsum pair):
    t[b, r]  = sum_{i,j,k} x[b,i,j,k] * f0[i,r] * f1[j,r] * f2[k,r]
    out[b,c] = sum_r t[b,r] * weight[r] * f3[c,r] + bias[0]

Strategy: data-parallel over the batch dim (32 batches per core, CP
factors replicated).  Per core the big contraction is restructured as
    z[r, b, k] = sum_{ij} (f0[i,r]*f1[j,r]*weight[r]) * x[b, ij, k]
which is a K=2304 matmul against the Khatri-Rao product of f0 and f1,
run as 18 K-chunks of 128 partitions at full PE rate (float32r).  The
remaining k-contraction against f2 runs on the vector engine, and the
class projection against f3^T is one small matmul.  x is pre-permuted
on the host so every DMA is 128 partitions x 6 KiB contiguous runs —
the kernel is HBM-bandwidth bound on loading x (~14.2 MB/core).
"""

import os

import numpy as np

_B, _M1, _M2, _M3, _C, _R = 256, 48, 48, 48, 1000, 64
_NCORES = 8
_BL = _B // _NCORES          # 32 batches per core
_IJ = _M1 * _M2              # 2304 contraction size (i,j fused)
_NCH = _IJ // 128            # 18 K-chunks of 128 partitions
_KB = _BL * _M3              # 1536 moving columns (b,k fused)
_SL = 512                    # matmul slice width (one PSUM bank, fp32)

_cache = {}


def _split_excess_waits(nc, mybir, max_waits=1):
    """Walrus in this container rejects >1 sync-wait per instruction
    ("Too many sync wait commands").  Move excess waits onto chained
    NoOps inserted just before the offending instruction (same engine,
    so program order preserves the gating)."""
    for bb in nc.m.functions[0].blocks:
        insts = bb.instructions
        i = 0
        while i < len(insts):
            inst = insts[i]
            si = getattr(inst, "sync_info", None)
            waits = list(si.on_wait) if si is not None and si.on_wait else []
            if len(waits) > max_waits:
                rest, keep = waits[:-max_waits], waits[-max_waits:]
                pos = i
                for j in range(0, len(rest), max_waits):
                    nop = mybir.InstNoOp(
                        name=f"I-waitsplit-{nc.next_id()}",
                        engine=inst.engine,
                        ins=[],
                        outs=[],
                        sync_info=mybir.SyncInfo(
                            on_wait=list(rest[j : j + max_waits]), on_update=[]
                        ),
                    )
                    nc.register_instruction(nop)
                    insts.insert(pos, nop)
                    pos += 1
                    i += 1
                si.on_wait = keep
            i += 1


def _bcast(ap, bass, shape3):
    """AP broadcast helper: make a 3D view with a stride-0 middle dim."""
    try:
        return ap.unsqueeze(1).broadcast_to(shape3)
    except Exception:
        a = ap.ap
        return bass.AP(
            tensor=ap.tensor,
            offset=ap.offset,
            ap=[list(a[0]), [0, shape3[1]], list(a[1])],
        )


def _build_program():
    import ml_dtypes
    import concourse.bass as bass
    import concourse.tile as tile
    from concourse import mybir

    f32 = mybir.dt.float32
    f32r = mybir.dt.float32r
    bf16 = mybir.dt.bfloat16

    nc = bass.Bass("TRN2", target_bir_lowering=False, debug=False,
                   num_devices=_NCORES)

    x_d = nc.dram_tensor("x", [128, _NCH, _BL, _M3], f32, kind="ExternalInput")
    f0t_d = nc.dram_tensor("f0t", [_R, _M1], f32, kind="ExternalInput")
    f1t_d = nc.dram_tensor("f1t", [_R, _M2], f32, kind="ExternalInput")
    f2t_d = nc.dram_tensor("f2t", [_R, _M3], f32, kind="ExternalInput")
    f3t_d = nc.dram_tensor("f3t", [_R, _C], f32r, kind="ExternalInput")
    w_d = nc.dram_tensor("w", [_R, 1], f32, kind="ExternalInput")
    b_d = nc.dram_tensor("b", [1, 1], f32, kind="ExternalInput")
    out_d = nc.dram_tensor("out", [_BL, _C], f32, kind="ExternalOutput")
    ident_d = nc.inline_tensor(
        np.eye(_R, dtype=np.float32).astype(ml_dtypes.bfloat16), name="ident64"
    )

    NGRP = 6                       # KR built in 6 groups of 8 i-rows
    GI = _M1 // NGRP               # 8 i-rows per group = 384 ij = 3 chunks
    HALF = _NCH // 2               # chunks 0-8 -> z_a, 9-17 -> z_b

    with tile.TileContext(nc) as tc:
        with (
            tc.tile_pool(name="consts", bufs=1) as consts,
            tc.tile_pool(name="xstgp", bufs=8) as xstgp,
            tc.tile_pool(name="xp", bufs=_NCH) as xp,
            tc.tile_pool(name="work", bufs=1) as work,
            tc.tile_pool(name="pz", bufs=1, space=bass.MemorySpace.PSUM) as pz,
        ):
            # ---- critical-path DMAs first: f0/f1/identity (sync ring) ----
            f0t = consts.tile([_R, _M1], f32)
            nc.sync.dma_start(out=f0t[:], in_=f0t_d[:])
            f1t = consts.tile([_R, _M2], f32)
            nc.sync.dma_start(out=f1t[:], in_=f1t_d[:])
            idn = consts.tile([_R, _R], bf16)
            nc.sync.dma_start(out=idn[:], in_=ident_d[:])

            # ---- x stream: HWDGE fp32 DMA (both rings) into staging slots,
            # cast fp32 -> bf16 split across DVE and ACT ----
            xms = []
            for m in range(_NCH):
                stg = xstgp.tile([128, _BL, _M3], f32, tag="xstg")
                dma_eng = nc.sync if m % 2 == 0 else nc.scalar
                dma_eng.dma_start(out=stg[:], in_=x_d[:, m])
                xm = xp.tile([128, _BL, _M3], bf16, tag="x")
                if m % 2 == 0:
                    nc.vector.tensor_copy(xm[:], stg[:])
                else:
                    nc.gpsimd.tensor_copy(xm[:], stg[:])
                xms.append(xm)

            # ---- non-critical constants (behind x on the rings) ----
            f2t = consts.tile([_R, _M3], f32)
            nc.sync.dma_start(out=f2t[:], in_=f2t_d[:])
            f3t = consts.tile([_R, _C], f32r)
            nc.sync.dma_start(out=f3t[:], in_=f3t_d[:])
            wsb = consts.tile([_R, 1], f32)
            nc.sync.dma_start(out=wsb[:], in_=w_d[:])
            bsb = consts.tile([_BL, 1], f32)
            b_ap = b_d[:]
            nc.gpsimd.dma_start(
                out=bsb[:],
                in_=bass.AP(tensor=b_ap.tensor, offset=b_ap.offset,
                            ap=[[0, _BL], [0, 1]]),
            )
            # weight folds into f2 (off the kr critical path)
            f2tw = consts.tile([_R, _M3], f32)
            nc.vector.tensor_scalar_mul(f2tw[:], f2t[:], wsb[:])

            # ---- KR = f0 (x) f1, built in groups, transposed to put ij on
            # partitions: kr[p, m, r] = KR[128m+p, r] ----
            krt = consts.tile([_R, _M1, _M2], bf16)
            kr = consts.tile([128, _NCH, _R], bf16)
            krt_flat = krt[:].rearrange("r i j -> r (i j)")
            with tc.tile_pool(
                name="pt", bufs=2, space=bass.MemorySpace.PSUM
            ) as pt:
                for g in range(NGRP):
                    i0 = g * GI
                    in0 = (
                        f0t[:, i0 : i0 + GI]
                        .unsqueeze(2)
                        .broadcast_to((_R, GI, _M2))
                    )
                    in1 = _bcast(f1t[:], bass, (_R, GI, _M2))
                    nc.vector.tensor_mul(krt[:, i0 : i0 + GI, :], in0, in1)
                    for mm in range(3):
                        m = 3 * g + mm
                        pkr = pt.tile([128, _R], bf16)
                        nc.tensor.transpose(
                            pkr[:], krt_flat[:, m * 128 : (m + 1) * 128], idn[:]
                        )
                        nc.scalar.copy(kr[:, m, :], pkr[:])

            # ---- main contraction, split into two accumulators so half the
            # k-contraction overlaps the stream ----
            za = pz.tile([_R, _KB], f32, tag="za")
            zb = pz.tile([_R, _KB], f32, tag="zb")
            f2b = _bcast(f2tw[:], bass, (_R, _BL, _M3))

            def emit_chunk(m, ztile, start, stop):
                xm_f = xms[m][:].rearrange("p b k -> p (b k)")
                for s in range(_KB // _SL):
                    nc.tensor.matmul(
                        ztile[:, s * _SL : (s + 1) * _SL],
                        lhsT=kr[:, m, :],
                        rhs=xm_f[:, s * _SL : (s + 1) * _SL],
                        start=start,
                        stop=stop,
                    )

            for m in range(HALF):
                emit_chunk(m, za, m == 0, m == HALF - 1)
            for m in range(HALF, _NCH):
                emit_chunk(m, zb, m == HALF, m == _NCH - 1)

            # k-contraction of the first half (can run mid-stream)
            zfa = work.tile([_R, _BL, _M3], f32, tag="zfa")
            nc.vector.tensor_mul(
                zfa[:], za[:].rearrange("r (b k) -> r b k", k=_M3), f2b
            )
            ta = work.tile([_R, _BL], f32, tag="ta")
            nc.vector.reduce_sum(ta[:], zfa[:], axis=mybir.AxisListType.X)

            zfb = work.tile([_R, _BL, _M3], f32, tag="zfb")
            nc.vector.tensor_mul(
                zfb[:], zb[:].rearrange("r (b k) -> r b k", k=_M3), f2b
            )
            tb = work.tile([_R, _BL], f32, tag="tb")
            nc.vector.reduce_sum(tb[:], zfb[:], axis=mybir.AxisListType.X)

            tsb = work.tile([_R, _BL], f32r, tag="tsb")
            with nc.allow_low_precision(reason="f32r rounding for PE matmul"):
                nc.vector.tensor_add(tsb[:], ta[:], tb[:])

            # ---- class projection + bias, pipelined by half ----
            osb = work.tile([_BL, _C], f32, tag="osb")
            with tc.tile_pool(
                name="po", bufs=1, space=bass.MemorySpace.PSUM
            ) as po:
                op = po.tile([_BL, _C], f32)
                for n0, n1 in ((0, _SL), (_SL, _C)):
                    nc.tensor.matmul(
                        op[:, n0:n1],
                        lhsT=tsb[:],
                        rhs=f3t[:, n0:n1],
                        start=True,
                        stop=True,
                    )
                    nc.scalar.add(osb[:, n0:n1], op[:, n0:n1], bsb[:])
                    nc.sync.dma_start(
                        out=out_d[:, n0:n1], in_=osb[:, n0:n1]
                    )

    _split_excess_waits(nc, mybir)
    return nc


def _get_program():
    if "nc" not in _cache:
        _cache["nc"] = _build_program()
    return _cache["nc"]


def _host_prep(x, weight, f0, f1, f2, f3, bias):
    """Shard x over cores (batch dim) in a DMA-friendly layout, and
    transpose the small factor matrices (layout only, plus reshapes)."""
    x = np.ascontiguousarray(np.asarray(x, dtype=np.float32))
    f0t = np.ascontiguousarray(np.asarray(f0, np.float32).T)
    f1t = np.ascontiguousarray(np.asarray(f1, np.float32).T)
    f2t = np.ascontiguousarray(np.asarray(f2, np.float32).T)
    f3t = np.ascontiguousarray(np.asarray(f3, np.float32).T)
    w = np.ascontiguousarray(np.asarray(weight, np.float32).reshape(_R, 1))
    b = np.ascontiguousarray(np.asarray(bias, np.float32).reshape(1, 1))
    in_maps = []
    for c in range(_NCORES):
        xc = x[c * _BL : (c + 1) * _BL]
        # [b, ij, k] -> [p, m, b, k] with ij = 128*m + p
        xd = np.ascontiguousarray(
            xc.reshape(_BL, _NCH, 128, _M3).transpose(2, 1, 0, 3)
        )
        in_maps.append(
            {"x": xd, "f0t": f0t, "f1t": f1t, "f2t": f2t, "f3t": f3t,
             "w": w, "b": b}
        )
    return in_maps


LAST_EXEC_NS = None


def kernel(x, weight, f0, f1, f2, f3, bias):
    global LAST_EXEC_NS
    from concourse.bass_utils import run_bass_kernel_spmd

    nc = _get_program()
    in_maps = _host_prep(x, weight, f0, f1, f2, f3, bias)
    trace = bool(int(os.environ.get("BASS_KERNEL_TRACE", "0")))
    res = run_bass_kernel_spmd(nc, in_maps, list(range(_NCORES)), trace=trace)
    LAST_EXEC_NS = res.exec_time_ns
    out = np.concatenate([res.results[c]["out"] for c in range(_NCORES)], axis=0)
    return np.ascontiguousarray(out.astype(np.float32, copy=False))


# revision 23
# speedup vs baseline: 1.0745x; 1.0745x over previous
"""CP tensor-regression-layer kernel for Trainium2 (8 NeuronCores).

Computation (matches the reference einsum pair):
    t[b, r]  = sum_{i,j,k} x[b,i,j,k] * f0[i,r] * f1[j,r] * f2[k,r]
    out[b,c] = sum_r t[b,r] * weight[r] * f3[c,r] + bias[0]

Strategy: data-parallel over the batch dim (32 batches per core, CP
factors replicated).  Per core the big contraction is restructured as
    z[r, b, k] = sum_{ij} (f0[i,r]*f1[j,r]*weight[r]) * x[b, ij, k]
which is a K=2304 matmul against the Khatri-Rao product of f0 and f1,
run as 18 K-chunks of 128 partitions at full PE rate (float32r).  The
remaining k-contraction against f2 runs on the vector engine, and the
class projection against f3^T is one small matmul.  x is pre-permuted
on the host so every DMA is 128 partitions x 6 KiB contiguous runs —
the kernel is HBM-bandwidth bound on loading x (~14.2 MB/core).
"""

import os

import numpy as np

_B, _M1, _M2, _M3, _C, _R = 256, 48, 48, 48, 1000, 64
_NCORES = 8
_BL = _B // _NCORES          # 32 batches per core
_IJ = _M1 * _M2              # 2304 contraction size (i,j fused)
_NCH = _IJ // 128            # 18 K-chunks of 128 partitions
_KB = _BL * _M3              # 1536 moving columns (b,k fused)
_SL = 512                    # matmul slice width (one PSUM bank, fp32)

_cache = {}


def _split_excess_waits(nc, mybir, max_waits=1):
    """Walrus in this container rejects >1 sync-wait per instruction
    ("Too many sync wait commands").  Move excess waits onto chained
    NoOps inserted just before the offending instruction (same engine,
    so program order preserves the gating)."""
    for bb in nc.m.functions[0].blocks:
        insts = bb.instructions
        i = 0
        while i < len(insts):
            inst = insts[i]
            si = getattr(inst, "sync_info", None)
            waits = list(si.on_wait) if si is not None and si.on_wait else []
            if len(waits) > max_waits:
                rest, keep = waits[:-max_waits], waits[-max_waits:]
                pos = i
                for j in range(0, len(rest), max_waits):
                    nop = mybir.InstNoOp(
                        name=f"I-waitsplit-{nc.next_id()}",
                        engine=inst.engine,
                        ins=[],
                        outs=[],
                        sync_info=mybir.SyncInfo(
                            on_wait=list(rest[j : j + max_waits]), on_update=[]
                        ),
                    )
                    nc.register_instruction(nop)
                    insts.insert(pos, nop)
                    pos += 1
                    i += 1
                si.on_wait = keep
            i += 1


def _bcast(ap, bass, shape3):
    """AP broadcast helper: make a 3D view with a stride-0 middle dim."""
    try:
        return ap.unsqueeze(1).broadcast_to(shape3)
    except Exception:
        a = ap.ap
        return bass.AP(
            tensor=ap.tensor,
            offset=ap.offset,
            ap=[list(a[0]), [0, shape3[1]], list(a[1])],
        )


def _build_program():
    import ml_dtypes
    import concourse.bass as bass
    import concourse.tile as tile
    from concourse import mybir

    f32 = mybir.dt.float32
    f32r = mybir.dt.float32r
    bf16 = mybir.dt.bfloat16

    nc = bass.Bass("TRN2", target_bir_lowering=False, debug=False,
                   num_devices=_NCORES)

    x_d = nc.dram_tensor("x", [128, _NCH, _BL, _M3], f32, kind="ExternalInput")
    f0t_d = nc.dram_tensor("f0t", [_R, _M1], f32, kind="ExternalInput")
    f1t_d = nc.dram_tensor("f1t", [_R, _M2], f32, kind="ExternalInput")
    f2t_d = nc.dram_tensor("f2t", [_R, _M3], f32, kind="ExternalInput")
    f3t_d = nc.dram_tensor("f3t", [_R, _C], f32r, kind="ExternalInput")
    w_d = nc.dram_tensor("w", [_R, 1], f32, kind="ExternalInput")
    b_d = nc.dram_tensor("b", [1, 1], f32, kind="ExternalInput")
    out_d = nc.dram_tensor("out", [_BL, _C], f32, kind="ExternalOutput")
    ident_d = nc.inline_tensor(
        np.eye(_R, dtype=np.float32).astype(ml_dtypes.bfloat16), name="ident64"
    )

    NGRP = 6                       # KR built in 6 groups of 8 i-rows
    GI = _M1 // NGRP               # 8 i-rows per group = 384 ij = 3 chunks
    HALF = _NCH // 2               # chunks 0-8 -> z_a, 9-17 -> z_b

    with tile.TileContext(nc) as tc:
        with (
            tc.tile_pool(name="consts", bufs=1) as consts,
            tc.tile_pool(name="xstgp", bufs=8) as xstgp,
            tc.tile_pool(name="xp", bufs=_NCH) as xp,
            tc.tile_pool(name="work", bufs=1) as work,
            tc.tile_pool(name="pz", bufs=1, space=bass.MemorySpace.PSUM) as pz,
        ):
            # ---- critical-path DMAs first: f0/f1/identity (sync ring) ----
            f0t = consts.tile([_R, _M1], f32)
            nc.sync.dma_start(out=f0t[:], in_=f0t_d[:])
            f1t = consts.tile([_R, _M2], f32)
            nc.sync.dma_start(out=f1t[:], in_=f1t_d[:])
            idn = consts.tile([_R, _R], bf16)
            nc.sync.dma_start(out=idn[:], in_=ident_d[:])

            # ---- x stream: HWDGE fp32 DMA (both rings) into staging slots,
            # cast fp32 -> bf16 split across DVE and ACT ----
            xms = []
            for m in range(_NCH):
                stg = xstgp.tile([128, _BL, _M3], f32, tag="xstg")
                dma_eng = nc.sync if m % 2 == 0 else nc.scalar
                dma_eng.dma_start(out=stg[:], in_=x_d[:, m])
                xm = xp.tile([128, _BL, _M3], bf16, tag="x")
                if m % 3 == 2:
                    nc.scalar.copy(xm[:], stg[:])
                else:
                    nc.vector.tensor_copy(xm[:], stg[:])
                xms.append(xm)

            # ---- non-critical constants (behind x on the rings) ----
            f2t = consts.tile([_R, _M3], f32)
            nc.sync.dma_start(out=f2t[:], in_=f2t_d[:])
            f3t = consts.tile([_R, _C], f32r)
            nc.sync.dma_start(out=f3t[:], in_=f3t_d[:])
            wsb = consts.tile([_R, 1], f32)
            nc.sync.dma_start(out=wsb[:], in_=w_d[:])
            bsb = consts.tile([_BL, 1], f32)
            b_ap = b_d[:]
            nc.gpsimd.dma_start(
                out=bsb[:],
                in_=bass.AP(tensor=b_ap.tensor, offset=b_ap.offset,
                            ap=[[0, _BL], [0, 1]]),
            )
            # weight folds into f2 (off the kr critical path)
            f2tw = consts.tile([_R, _M3], f32)
            nc.vector.tensor_scalar_mul(f2tw[:], f2t[:], wsb[:])

            # ---- KR = f0 (x) f1, built in groups, transposed to put ij on
            # partitions: kr[p, m, r] = KR[128m+p, r] ----
            krt = consts.tile([_R, _M1, _M2], bf16)
            kr = consts.tile([128, _NCH, _R], bf16)
            krt_flat = krt[:].rearrange("r i j -> r (i j)")
            with tc.tile_pool(
                name="pt", bufs=2, space=bass.MemorySpace.PSUM
            ) as pt:
                for g in range(NGRP):
                    i0 = g * GI
                    in0 = (
                        f0t[:, i0 : i0 + GI]
                        .unsqueeze(2)
                        .broadcast_to((_R, GI, _M2))
                    )
                    in1 = _bcast(f1t[:], bass, (_R, GI, _M2))
                    nc.vector.tensor_mul(krt[:, i0 : i0 + GI, :], in0, in1)
                    for mm in range(3):
                        m = 3 * g + mm
                        pkr = pt.tile([128, _R], bf16)
                        nc.tensor.transpose(
                            pkr[:], krt_flat[:, m * 128 : (m + 1) * 128], idn[:]
                        )
                        nc.scalar.copy(kr[:, m, :], pkr[:])

            # ---- main contraction, split into two accumulators so half the
            # k-contraction overlaps the stream ----
            za = pz.tile([_R, _KB], f32, tag="za")
            zb = pz.tile([_R, _KB], f32, tag="zb")
            f2b = _bcast(f2tw[:], bass, (_R, _BL, _M3))

            def emit_chunk(m, ztile, start, stop):
                xm_f = xms[m][:].rearrange("p b k -> p (b k)")
                for s in range(_KB // _SL):
                    nc.tensor.matmul(
                        ztile[:, s * _SL : (s + 1) * _SL],
                        lhsT=kr[:, m, :],
                        rhs=xm_f[:, s * _SL : (s + 1) * _SL],
                        start=start,
                        stop=stop,
                    )

            for m in range(HALF):
                emit_chunk(m, za, m == 0, m == HALF - 1)
            for m in range(HALF, _NCH):
                emit_chunk(m, zb, m == HALF, m == _NCH - 1)

            # k-contraction of the first half (can run mid-stream)
            zfa = work.tile([_R, _BL, _M3], f32, tag="zfa")
            nc.vector.tensor_mul(
                zfa[:], za[:].rearrange("r (b k) -> r b k", k=_M3), f2b
            )
            ta = work.tile([_R, _BL], f32, tag="ta")
            nc.vector.reduce_sum(ta[:], zfa[:], axis=mybir.AxisListType.X)

            zfb = work.tile([_R, _BL, _M3], f32, tag="zfb")
            nc.vector.tensor_mul(
                zfb[:], zb[:].rearrange("r (b k) -> r b k", k=_M3), f2b
            )
            tb = work.tile([_R, _BL], f32, tag="tb")
            nc.vector.reduce_sum(tb[:], zfb[:], axis=mybir.AxisListType.X)

            tsb = work.tile([_R, _BL], f32r, tag="tsb")
            with nc.allow_low_precision(reason="f32r rounding for PE matmul"):
                nc.vector.tensor_add(tsb[:], ta[:], tb[:])

            # ---- class projection + bias, pipelined by half ----
            osb = work.tile([_BL, _C], f32, tag="osb")
            with tc.tile_pool(
                name="po", bufs=1, space=bass.MemorySpace.PSUM
            ) as po:
                op = po.tile([_BL, _C], f32)
                for n0, n1 in ((0, _SL), (_SL, _C)):
                    nc.tensor.matmul(
                        op[:, n0:n1],
                        lhsT=tsb[:],
                        rhs=f3t[:, n0:n1],
                        start=True,
                        stop=True,
                    )
                    nc.scalar.add(osb[:, n0:n1], op[:, n0:n1], bsb[:])
                    nc.sync.dma_start(
                        out=out_d[:, n0:n1], in_=osb[:, n0:n1]
                    )

    _split_excess_waits(nc, mybir)
    return nc


def _get_program():
    if "nc" not in _cache:
        _cache["nc"] = _build_program()
    return _cache["nc"]


def _host_prep(x, weight, f0, f1, f2, f3, bias):
    """Shard x over cores (batch dim) in a DMA-friendly layout, and
    transpose the small factor matrices (layout only, plus reshapes)."""
    x = np.ascontiguousarray(np.asarray(x, dtype=np.float32))
    f0t = np.ascontiguousarray(np.asarray(f0, np.float32).T)
    f1t = np.ascontiguousarray(np.asarray(f1, np.float32).T)
    f2t = np.ascontiguousarray(np.asarray(f2, np.float32).T)
    f3t = np.ascontiguousarray(np.asarray(f3, np.float32).T)
    w = np.ascontiguousarray(np.asarray(weight, np.float32).reshape(_R, 1))
    b = np.ascontiguousarray(np.asarray(bias, np.float32).reshape(1, 1))
    in_maps = []
    for c in range(_NCORES):
        xc = x[c * _BL : (c + 1) * _BL]
        # [b, ij, k] -> [p, m, b, k] with ij = 128*m + p
        xd = np.ascontiguousarray(
            xc.reshape(_BL, _NCH, 128, _M3).transpose(2, 1, 0, 3)
        )
        in_maps.append(
            {"x": xd, "f0t": f0t, "f1t": f1t, "f2t": f2t, "f3t": f3t,
             "w": w, "b": b}
        )
    return in_maps


LAST_EXEC_NS = None


def kernel(x, weight, f0, f1, f2, f3, bias):
    global LAST_EXEC_NS
    from concourse.bass_utils import run_bass_kernel_spmd

    nc = _get_program()
    in_maps = _host_prep(x, weight, f0, f1, f2, f3, bias)
    trace = bool(int(os.environ.get("BASS_KERNEL_TRACE", "0")))
    res = run_bass_kernel_spmd(nc, in_maps, list(range(_NCORES)), trace=trace)
    LAST_EXEC_NS = res.exec_time_ns
    out = np.concatenate([res.results[c]["out"] for c in range(_NCORES)], axis=0)
    return np.ascontiguousarray(out.astype(np.float32, copy=False))


# revision 24
# speedup vs baseline: 1.2600x; 1.1727x over previous
"""CP tensor-regression-layer kernel for Trainium2 (8 NeuronCores).

Computation (matches the reference einsum pair):
    t[b, r]  = sum_{i,j,k} x[b,i,j,k] * f0[i,r] * f1[j,r] * f2[k,r]
    out[b,c] = sum_r t[b,r] * weight[r] * f3[c,r] + bias[0]

Strategy: data-parallel over the batch dim (32 batches per core, CP
factors replicated).  Per core the big contraction is restructured as
    z[r, b, k] = sum_{ij} (f0[i,r]*f1[j,r]*weight[r]) * x[b, ij, k]
which is a K=2304 matmul against the Khatri-Rao product of f0 and f1,
run as 18 K-chunks of 128 partitions at full PE rate (float32r).  The
remaining k-contraction against f2 runs on the vector engine, and the
class projection against f3^T is one small matmul.  x is pre-permuted
on the host so every DMA is 128 partitions x 6 KiB contiguous runs —
the kernel is HBM-bandwidth bound on loading x (~14.2 MB/core).
"""

import os

import numpy as np

_B, _M1, _M2, _M3, _C, _R = 256, 48, 48, 48, 1000, 64
_NCORES = 8
_BL = _B // _NCORES          # 32 batches per core
_IJ = _M1 * _M2              # 2304 contraction size (i,j fused)
_NCH = _IJ // 128            # 18 K-chunks of 128 partitions
_KB = _BL * _M3              # 1536 moving columns (b,k fused)
_SL = 512                    # matmul slice width (one PSUM bank, fp32)

_cache = {}


def _split_excess_waits(nc, mybir, max_waits=1):
    """Walrus in this container rejects >1 sync-wait per instruction
    ("Too many sync wait commands").  Move excess waits onto chained
    NoOps inserted just before the offending instruction (same engine,
    so program order preserves the gating)."""
    for bb in nc.m.functions[0].blocks:
        insts = bb.instructions
        i = 0
        while i < len(insts):
            inst = insts[i]
            si = getattr(inst, "sync_info", None)
            waits = list(si.on_wait) if si is not None and si.on_wait else []
            if len(waits) > max_waits:
                rest, keep = waits[:-max_waits], waits[-max_waits:]
                pos = i
                for j in range(0, len(rest), max_waits):
                    nop = mybir.InstNoOp(
                        name=f"I-waitsplit-{nc.next_id()}",
                        engine=inst.engine,
                        ins=[],
                        outs=[],
                        sync_info=mybir.SyncInfo(
                            on_wait=list(rest[j : j + max_waits]), on_update=[]
                        ),
                    )
                    nc.register_instruction(nop)
                    insts.insert(pos, nop)
                    pos += 1
                    i += 1
                si.on_wait = keep
            i += 1


def _bcast(ap, bass, shape3):
    """AP broadcast helper: make a 3D view with a stride-0 middle dim."""
    try:
        return ap.unsqueeze(1).broadcast_to(shape3)
    except Exception:
        a = ap.ap
        return bass.AP(
            tensor=ap.tensor,
            offset=ap.offset,
            ap=[list(a[0]), [0, shape3[1]], list(a[1])],
        )


def _build_program():
    import ml_dtypes
    import concourse.bass as bass
    import concourse.tile as tile
    from concourse import mybir

    f32 = mybir.dt.float32
    f32r = mybir.dt.float32r
    bf16 = mybir.dt.bfloat16

    nc = bass.Bass("TRN2", target_bir_lowering=False, debug=False,
                   num_devices=_NCORES)

    x_d = nc.dram_tensor("x", [128, _NCH, _BL, _M3], f32, kind="ExternalInput")
    f0t_d = nc.dram_tensor("f0t", [_R, _M1], f32, kind="ExternalInput")
    f1t_d = nc.dram_tensor("f1t", [_R, _M2], f32, kind="ExternalInput")
    f2t_d = nc.dram_tensor("f2t", [_R, _M3], f32, kind="ExternalInput")
    f3t_d = nc.dram_tensor("f3t", [_R, _C], f32r, kind="ExternalInput")
    w_d = nc.dram_tensor("w", [_R, 1], f32, kind="ExternalInput")
    b_d = nc.dram_tensor("b", [1, 1], f32, kind="ExternalInput")
    out_d = nc.dram_tensor("out", [_BL, _C], f32, kind="ExternalOutput")
    ident_d = nc.inline_tensor(
        np.eye(_R, dtype=np.float32).astype(ml_dtypes.bfloat16), name="ident64"
    )

    NGRP = 6                       # KR built in 6 groups of 8 i-rows
    GI = _M1 // NGRP               # 8 i-rows per group = 384 ij = 3 chunks
    HALF = _NCH // 2               # chunks 0-8 -> z_a, 9-17 -> z_b

    with tile.TileContext(nc) as tc:
        with (
            tc.tile_pool(name="consts", bufs=1) as consts,
            tc.tile_pool(name="xstgp", bufs=8) as xstgp,
            tc.tile_pool(name="xp", bufs=_NCH) as xp,
            tc.tile_pool(name="work", bufs=1) as work,
            tc.tile_pool(name="pz", bufs=1, space=bass.MemorySpace.PSUM) as pz,
        ):
            # ---- critical-path DMAs first: f0/f1/identity (sync ring) ----
            f0t = consts.tile([_R, _M1], f32)
            nc.sync.dma_start(out=f0t[:], in_=f0t_d[:])
            f1t = consts.tile([_R, _M2], f32)
            nc.sync.dma_start(out=f1t[:], in_=f1t_d[:])
            idn = consts.tile([_R, _R], bf16)
            nc.sync.dma_start(out=idn[:], in_=ident_d[:])

            # ---- KR = f0 (x) f1, built in groups, transposed to put ij on
            # partitions: kr[p, m, r] = KR[128m+p, r] ----
            krt = consts.tile([_R, _M1, _M2], bf16)
            kr = consts.tile([128, _NCH, _R], bf16)
            krt_flat = krt[:].rearrange("r i j -> r (i j)")
            with tc.tile_pool(
                name="pt", bufs=2, space=bass.MemorySpace.PSUM
            ) as pt:
                for g in range(NGRP):
                    i0 = g * GI
                    in0 = (
                        f0t[:, i0 : i0 + GI]
                        .unsqueeze(2)
                        .broadcast_to((_R, GI, _M2))
                    )
                    in1 = _bcast(f1t[:], bass, (_R, GI, _M2))
                    nc.vector.tensor_mul(krt[:, i0 : i0 + GI, :], in0, in1)
                    for mm in range(3):
                        m = 3 * g + mm
                        pkr = pt.tile([128, _R], bf16)
                        nc.tensor.transpose(
                            pkr[:], krt_flat[:, m * 128 : (m + 1) * 128], idn[:]
                        )
                        nc.scalar.copy(kr[:, m, :], pkr[:])

            # ---- x stream: HWDGE fp32 DMA (both rings) into staging slots,
            # cast fp32 -> bf16 split across DVE and ACT ----
            xms = []
            for m in range(_NCH):
                stg = xstgp.tile([128, _BL, _M3], f32, tag="xstg")
                dma_eng = nc.sync if m % 2 == 0 else nc.scalar
                dma_eng.dma_start(out=stg[:], in_=x_d[:, m])
                xm = xp.tile([128, _BL, _M3], bf16, tag="x")
                if m % 3 == 2:
                    nc.scalar.copy(xm[:], stg[:])
                else:
                    nc.vector.tensor_copy(xm[:], stg[:])
                xms.append(xm)

            # ---- non-critical constants (behind x on the rings) ----
            f2t = consts.tile([_R, _M3], f32)
            nc.sync.dma_start(out=f2t[:], in_=f2t_d[:])
            f3t = consts.tile([_R, _C], f32r)
            nc.sync.dma_start(out=f3t[:], in_=f3t_d[:])
            wsb = consts.tile([_R, 1], f32)
            nc.sync.dma_start(out=wsb[:], in_=w_d[:])
            bsb = consts.tile([_BL, 1], f32)
            b_ap = b_d[:]
            nc.gpsimd.dma_start(
                out=bsb[:],
                in_=bass.AP(tensor=b_ap.tensor, offset=b_ap.offset,
                            ap=[[0, _BL], [0, 1]]),
            )
            # weight folds into f2 (off the kr critical path)
            f2tw = consts.tile([_R, _M3], f32)
            nc.vector.tensor_scalar_mul(f2tw[:], f2t[:], wsb[:])

            # ---- main contraction, split into two accumulators so half the
            # k-contraction overlaps the stream ----
            za = pz.tile([_R, _KB], f32, tag="za")
            zb = pz.tile([_R, _KB], f32, tag="zb")
            f2b = _bcast(f2tw[:], bass, (_R, _BL, _M3))

            def emit_chunk(m, ztile, start, stop):
                xm_f = xms[m][:].rearrange("p b k -> p (b k)")
                for s in range(_KB // _SL):
                    nc.tensor.matmul(
                        ztile[:, s * _SL : (s + 1) * _SL],
                        lhsT=kr[:, m, :],
                        rhs=xm_f[:, s * _SL : (s + 1) * _SL],
                        start=start,
                        stop=stop,
                    )

            for m in range(HALF):
                emit_chunk(m, za, m == 0, m == HALF - 1)
            for m in range(HALF, _NCH):
                emit_chunk(m, zb, m == HALF, m == _NCH - 1)

            # k-contraction of the first half (can run mid-stream)
            zfa = work.tile([_R, _BL, _M3], f32, tag="zfa")
            nc.vector.tensor_mul(
                zfa[:], za[:].rearrange("r (b k) -> r b k", k=_M3), f2b
            )
            ta = work.tile([_R, _BL], f32, tag="ta")
            nc.vector.reduce_sum(ta[:], zfa[:], axis=mybir.AxisListType.X)

            zfb = work.tile([_R, _BL, _M3], f32, tag="zfb")
            nc.vector.tensor_mul(
                zfb[:], zb[:].rearrange("r (b k) -> r b k", k=_M3), f2b
            )
            tb = work.tile([_R, _BL], f32, tag="tb")
            nc.vector.reduce_sum(tb[:], zfb[:], axis=mybir.AxisListType.X)

            tsb = work.tile([_R, _BL], f32r, tag="tsb")
            with nc.allow_low_precision(reason="f32r rounding for PE matmul"):
                nc.vector.tensor_add(tsb[:], ta[:], tb[:])

            # ---- class projection + bias, pipelined by half ----
            osb = work.tile([_BL, _C], f32, tag="osb")
            with tc.tile_pool(
                name="po", bufs=1, space=bass.MemorySpace.PSUM
            ) as po:
                op = po.tile([_BL, _C], f32)
                for n0, n1 in ((0, _SL), (_SL, _C)):
                    nc.tensor.matmul(
                        op[:, n0:n1],
                        lhsT=tsb[:],
                        rhs=f3t[:, n0:n1],
                        start=True,
                        stop=True,
                    )
                    nc.scalar.add(osb[:, n0:n1], op[:, n0:n1], bsb[:])
                    nc.sync.dma_start(
                        out=out_d[:, n0:n1], in_=osb[:, n0:n1]
                    )

    _split_excess_waits(nc, mybir)
    return nc


def _get_program():
    if "nc" not in _cache:
        _cache["nc"] = _build_program()
    return _cache["nc"]


def _host_prep(x, weight, f0, f1, f2, f3, bias):
    """Shard x over cores (batch dim) in a DMA-friendly layout, and
    transpose the small factor matrices (layout only, plus reshapes)."""
    x = np.ascontiguousarray(np.asarray(x, dtype=np.float32))
    f0t = np.ascontiguousarray(np.asarray(f0, np.float32).T)
    f1t = np.ascontiguousarray(np.asarray(f1, np.float32).T)
    f2t = np.ascontiguousarray(np.asarray(f2, np.float32).T)
    f3t = np.ascontiguousarray(np.asarray(f3, np.float32).T)
    w = np.ascontiguousarray(np.asarray(weight, np.float32).reshape(_R, 1))
    b = np.ascontiguousarray(np.asarray(bias, np.float32).reshape(1, 1))
    in_maps = []
    for c in range(_NCORES):
        xc = x[c * _BL : (c + 1) * _BL]
        # [b, ij, k] -> [p, m, b, k] with ij = 128*m + p
        xd = np.ascontiguousarray(
            xc.reshape(_BL, _NCH, 128, _M3).transpose(2, 1, 0, 3)
        )
        in_maps.append(
            {"x": xd, "f0t": f0t, "f1t": f1t, "f2t": f2t, "f3t": f3t,
             "w": w, "b": b}
        )
    return in_maps


LAST_EXEC_NS = None


def kernel(x, weight, f0, f1, f2, f3, bias):
    global LAST_EXEC_NS
    from concourse.bass_utils import run_bass_kernel_spmd

    nc = _get_program()
    in_maps = _host_prep(x, weight, f0, f1, f2, f3, bias)
    trace = bool(int(os.environ.get("BASS_KERNEL_TRACE", "0")))
    res = run_bass_kernel_spmd(nc, in_maps, list(range(_NCORES)), trace=trace)
    LAST_EXEC_NS = res.exec_time_ns
    out = np.concatenate([res.results[c]["out"] for c in range(_NCORES)], axis=0)
    return np.ascontiguousarray(out.astype(np.float32, copy=False))


# revision 25
# speedup vs baseline: 1.3903x; 1.1034x over previous
"""CP tensor-regression-layer kernel for Trainium2 (8 NeuronCores).

Computation (matches the reference einsum pair):
    t[b, r]  = sum_{i,j,k} x[b,i,j,k] * f0[i,r] * f1[j,r] * f2[k,r]
    out[b,c] = sum_r t[b,r] * weight[r] * f3[c,r] + bias[0]

Strategy: data-parallel over the batch dim (32 batches per core, CP
factors replicated).  Per core the big contraction is restructured as
    z[r, b, k] = sum_{ij} (f0[i,r]*f1[j,r]*weight[r]) * x[b, ij, k]
which is a K=2304 matmul against the Khatri-Rao product of f0 and f1,
run as 18 K-chunks of 128 partitions at full PE rate (float32r).  The
remaining k-contraction against f2 runs on the vector engine, and the
class projection against f3^T is one small matmul.  x is pre-permuted
on the host so every DMA is 128 partitions x 6 KiB contiguous runs —
the kernel is HBM-bandwidth bound on loading x (~14.2 MB/core).
"""

import os

import numpy as np

_B, _M1, _M2, _M3, _C, _R = 256, 48, 48, 48, 1000, 64
_NCORES = 8
_BL = _B // _NCORES          # 32 batches per core
_IJ = _M1 * _M2              # 2304 contraction size (i,j fused)
_NCH = _IJ // 128            # 18 K-chunks of 128 partitions
_KB = _BL * _M3              # 1536 moving columns (b,k fused)
_SL = 512                    # matmul slice width (one PSUM bank, fp32)

_cache = {}


def _split_excess_waits(nc, mybir, max_waits=1):
    """Walrus in this container rejects >1 sync-wait per instruction
    ("Too many sync wait commands").  Move excess waits onto chained
    NoOps inserted just before the offending instruction (same engine,
    so program order preserves the gating)."""
    for bb in nc.m.functions[0].blocks:
        insts = bb.instructions
        i = 0
        while i < len(insts):
            inst = insts[i]
            si = getattr(inst, "sync_info", None)
            waits = list(si.on_wait) if si is not None and si.on_wait else []
            if len(waits) > max_waits:
                rest, keep = waits[:-max_waits], waits[-max_waits:]
                pos = i
                for j in range(0, len(rest), max_waits):
                    nop = mybir.InstNoOp(
                        name=f"I-waitsplit-{nc.next_id()}",
                        engine=inst.engine,
                        ins=[],
                        outs=[],
                        sync_info=mybir.SyncInfo(
                            on_wait=list(rest[j : j + max_waits]), on_update=[]
                        ),
                    )
                    nc.register_instruction(nop)
                    insts.insert(pos, nop)
                    pos += 1
                    i += 1
                si.on_wait = keep
            i += 1


def _bcast(ap, bass, shape3):
    """AP broadcast helper: make a 3D view with a stride-0 middle dim."""
    try:
        return ap.unsqueeze(1).broadcast_to(shape3)
    except Exception:
        a = ap.ap
        return bass.AP(
            tensor=ap.tensor,
            offset=ap.offset,
            ap=[list(a[0]), [0, shape3[1]], list(a[1])],
        )


def _build_program():
    import ml_dtypes
    import concourse.bass as bass
    import concourse.tile as tile
    from concourse import mybir

    f32 = mybir.dt.float32
    f32r = mybir.dt.float32r
    bf16 = mybir.dt.bfloat16

    nc = bass.Bass("TRN2", target_bir_lowering=False, debug=False,
                   num_devices=_NCORES)

    x_d = nc.dram_tensor("x", [128, _NCH, _BL, _M3], f32, kind="ExternalInput")
    f0t_d = nc.dram_tensor("f0t", [_R, _M1], f32, kind="ExternalInput")
    f1t_d = nc.dram_tensor("f1t", [_R, _M2], f32, kind="ExternalInput")
    f2t_d = nc.dram_tensor("f2t", [_R, _M3], f32, kind="ExternalInput")
    f3t_d = nc.dram_tensor("f3t", [_R, _C], f32r, kind="ExternalInput")
    w_d = nc.dram_tensor("w", [_R, 1], f32, kind="ExternalInput")
    b_d = nc.dram_tensor("b", [1, 1], f32, kind="ExternalInput")
    out_d = nc.dram_tensor("out", [_BL, _C], f32, kind="ExternalOutput")
    ident_d = nc.inline_tensor(
        np.eye(_R, dtype=np.float32).astype(ml_dtypes.bfloat16), name="ident64"
    )

    NGRP = 6                       # KR built in 6 groups of 8 i-rows
    GI = _M1 // NGRP               # 8 i-rows per group = 384 ij = 3 chunks
    HALF = _NCH // 2               # chunks 0-8 -> z_a, 9-17 -> z_b

    with tile.TileContext(nc) as tc:
        with (
            tc.tile_pool(name="consts", bufs=1) as consts,
            tc.tile_pool(name="xstgp", bufs=8) as xstgp,
            tc.tile_pool(name="xp", bufs=_NCH) as xp,
            tc.tile_pool(name="work", bufs=1) as work,
            tc.tile_pool(name="pz", bufs=1, space=bass.MemorySpace.PSUM) as pz,
        ):
            # ---- critical-path DMAs first: f0/f1/identity (sync ring) ----
            f0t = consts.tile([_R, _M1], f32)
            nc.sync.dma_start(out=f0t[:], in_=f0t_d[:])
            f1t = consts.tile([_R, _M2], f32)
            nc.sync.dma_start(out=f1t[:], in_=f1t_d[:])
            idn = consts.tile([_R, _R], bf16)
            nc.sync.dma_start(out=idn[:], in_=ident_d[:])

            # ---- KR = f0 (x) f1 (transposed so ij lands on partitions:
            # kr[p, m, r] = KR[128m+p, r]), interleaved with the x stream.
            # Each group g builds kr for chunks 3g..3g+2, emitted right
            # before those chunks' DMAs+casts: DVE does the kr work while
            # waiting on staging DMAs, and the DMA-issuing engines (SP for
            # even chunks, ACT for odd) never sit behind PSUM copies. ----
            krt = consts.tile([_R, _M1, _M2], bf16)
            kr = consts.tile([128, _NCH, _R], bf16)
            krt_flat = krt[:].rearrange("r i j -> r (i j)")
            xms = []
            with tc.tile_pool(
                name="pt", bufs=2, space=bass.MemorySpace.PSUM
            ) as pt:
                for g in range(NGRP):
                    i0 = g * GI
                    in0 = (
                        f0t[:, i0 : i0 + GI]
                        .unsqueeze(2)
                        .broadcast_to((_R, GI, _M2))
                    )
                    in1 = _bcast(f1t[:], bass, (_R, GI, _M2))
                    nc.vector.tensor_mul(krt[:, i0 : i0 + GI, :], in0, in1)
                    for mm in range(3):
                        m = 3 * g + mm
                        pkr = pt.tile([128, _R], bf16)
                        nc.tensor.transpose(
                            pkr[:], krt_flat[:, m * 128 : (m + 1) * 128], idn[:]
                        )
                        nc.vector.tensor_copy(kr[:, m, :], pkr[:])
                        # chunk m of the x stream
                        stg = xstgp.tile([128, _BL, _M3], f32, tag="xstg")
                        dma_eng = nc.sync if m % 2 == 0 else nc.scalar
                        dma_eng.dma_start(out=stg[:], in_=x_d[:, m])
                        xm = xp.tile([128, _BL, _M3], bf16, tag="x")
                        if m % 3 == 2:
                            nc.scalar.copy(xm[:], stg[:])
                        else:
                            nc.vector.tensor_copy(xm[:], stg[:])
                        xms.append(xm)

            # ---- non-critical constants (behind x on the rings) ----
            f2t = consts.tile([_R, _M3], f32)
            nc.sync.dma_start(out=f2t[:], in_=f2t_d[:])
            f3t = consts.tile([_R, _C], f32r)
            nc.sync.dma_start(out=f3t[:], in_=f3t_d[:])
            wsb = consts.tile([_R, 1], f32)
            nc.sync.dma_start(out=wsb[:], in_=w_d[:])
            bsb = consts.tile([_BL, 1], f32)
            b_ap = b_d[:]
            nc.gpsimd.dma_start(
                out=bsb[:],
                in_=bass.AP(tensor=b_ap.tensor, offset=b_ap.offset,
                            ap=[[0, _BL], [0, 1]]),
            )
            # weight folds into f2 (off the kr critical path)
            f2tw = consts.tile([_R, _M3], f32)
            nc.vector.tensor_scalar_mul(f2tw[:], f2t[:], wsb[:])

            # ---- main contraction, split into two accumulators so half the
            # k-contraction overlaps the stream ----
            za = pz.tile([_R, _KB], f32, tag="za")
            zb = pz.tile([_R, _KB], f32, tag="zb")
            f2b = _bcast(f2tw[:], bass, (_R, _BL, _M3))

            def emit_chunk(m, ztile, start, stop):
                xm_f = xms[m][:].rearrange("p b k -> p (b k)")
                for s in range(_KB // _SL):
                    nc.tensor.matmul(
                        ztile[:, s * _SL : (s + 1) * _SL],
                        lhsT=kr[:, m, :],
                        rhs=xm_f[:, s * _SL : (s + 1) * _SL],
                        start=start,
                        stop=stop,
                    )

            for m in range(HALF):
                emit_chunk(m, za, m == 0, m == HALF - 1)
            for m in range(HALF, _NCH):
                emit_chunk(m, zb, m == HALF, m == _NCH - 1)

            # k-contraction of the first half (can run mid-stream)
            zfa = work.tile([_R, _BL, _M3], f32, tag="zfa")
            nc.vector.tensor_mul(
                zfa[:], za[:].rearrange("r (b k) -> r b k", k=_M3), f2b
            )
            ta = work.tile([_R, _BL], f32, tag="ta")
            nc.vector.reduce_sum(ta[:], zfa[:], axis=mybir.AxisListType.X)

            zfb = work.tile([_R, _BL, _M3], f32, tag="zfb")
            nc.vector.tensor_mul(
                zfb[:], zb[:].rearrange("r (b k) -> r b k", k=_M3), f2b
            )
            tb = work.tile([_R, _BL], f32, tag="tb")
            nc.vector.reduce_sum(tb[:], zfb[:], axis=mybir.AxisListType.X)

            tsb = work.tile([_R, _BL], f32r, tag="tsb")
            with nc.allow_low_precision(reason="f32r rounding for PE matmul"):
                nc.vector.tensor_add(tsb[:], ta[:], tb[:])

            # ---- class projection + bias, pipelined by half ----
            osb = work.tile([_BL, _C], f32, tag="osb")
            with tc.tile_pool(
                name="po", bufs=1, space=bass.MemorySpace.PSUM
            ) as po:
                op = po.tile([_BL, _C], f32)
                for n0, n1 in ((0, _SL), (_SL, _C)):
                    nc.tensor.matmul(
                        op[:, n0:n1],
                        lhsT=tsb[:],
                        rhs=f3t[:, n0:n1],
                        start=True,
                        stop=True,
                    )
                    nc.scalar.add(osb[:, n0:n1], op[:, n0:n1], bsb[:])
                    nc.sync.dma_start(
                        out=out_d[:, n0:n1], in_=osb[:, n0:n1]
                    )

    _split_excess_waits(nc, mybir)
    return nc


def _get_program():
    if "nc" not in _cache:
        _cache["nc"] = _build_program()
    return _cache["nc"]


def _host_prep(x, weight, f0, f1, f2, f3, bias):
    """Shard x over cores (batch dim) in a DMA-friendly layout, and
    transpose the small factor matrices (layout only, plus reshapes)."""
    x = np.ascontiguousarray(np.asarray(x, dtype=np.float32))
    f0t = np.ascontiguousarray(np.asarray(f0, np.float32).T)
    f1t = np.ascontiguousarray(np.asarray(f1, np.float32).T)
    f2t = np.ascontiguousarray(np.asarray(f2, np.float32).T)
    f3t = np.ascontiguousarray(np.asarray(f3, np.float32).T)
    w = np.ascontiguousarray(np.asarray(weight, np.float32).reshape(_R, 1))
    b = np.ascontiguousarray(np.asarray(bias, np.float32).reshape(1, 1))
    in_maps = []
    for c in range(_NCORES):
        xc = x[c * _BL : (c + 1) * _BL]
        # [b, ij, k] -> [p, m, b, k] with ij = 128*m + p
        xd = np.ascontiguousarray(
            xc.reshape(_BL, _NCH, 128, _M3).transpose(2, 1, 0, 3)
        )
        in_maps.append(
            {"x": xd, "f0t": f0t, "f1t": f1t, "f2t": f2t, "f3t": f3t,
             "w": w, "b": b}
        )
    return in_maps


LAST_EXEC_NS = None


def kernel(x, weight, f0, f1, f2, f3, bias):
    global LAST_EXEC_NS
    from concourse.bass_utils import run_bass_kernel_spmd

    nc = _get_program()
    in_maps = _host_prep(x, weight, f0, f1, f2, f3, bias)
    trace = bool(int(os.environ.get("BASS_KERNEL_TRACE", "0")))
    res = run_bass_kernel_spmd(nc, in_maps, list(range(_NCORES)), trace=trace)
    LAST_EXEC_NS = res.exec_time_ns
    out = np.concatenate([res.results[c]["out"] for c in range(_NCORES)], axis=0)
    return np.ascontiguousarray(out.astype(np.float32, copy=False))
